# revision 19
# baseline (speedup 1.0000x reference)
"""Trainium2 Bass kernel for masked graph-convolution interaction.

Math (reference):
    wf = node_features @ weight                              # [N, D]
    T[i,d,j] = wf[i,d] * wf[j,d] * mh[i,j]
    S[a,d,j] = sum_i adj[a,i] * T[i,d,j]
    out[a,d] = sum_j S[a,d,j] * mf[a,j] / ncnt[a]^2

fp8 scheme (mean-centered so e4m3 quantization error stays ~6e-3):
    mh = 0.5 + mh',  adj = 0.5 + adj',  mfs = mf / ncnt^2 (folded on host)
    X'_a[i,d] = adj'[a,i] * wf[i,d]                 (fp8, DVE/ACT scale-copy)
    Y'_a = mh'8^T @ X'8_a                           (PE, fp8 DoubleRow = 2x)
    out[a,d] = sum_j mfs[a,j] wf[j,d] Y'_a[j,d] + C[a,d]
    C = 0.5*(adj@wf) .* (mfs@wf) + 0.5*mfs@(wf .* (mh'^T@wf))   (exact, tiny)

Stage 2: ACT drains Y' PSUM->SBUF bf16, DVE multiplies by wf (bf16 2x mode),
per-row j-contraction as 4-way col-tiled PE matvecs (lhsT = mfs column), and
the C correction is accumulated into the same PSUM via one-hot id columns.

Sharding: row-split of a across 8 cores (128 rows each); mh / wf replicated.
"""

import numpy as np

N = 1024
DIN = 256
DOUT = 128
NCORES = 8
ROWS = N // NCORES  # 128 output rows per core
P = 128
IC = N // P  # 8 chunks over i
ICP = IC // 2  # 4 DoubleRow pairs
JC = N // P  # 8 chunks over j
KC = DIN // P  # 2 chunks over k (wf compute)
G4 = 4  # rows per group
NG = ROWS // G4  # 32 groups per core

# how many of the 8 per-group X' ic-chunks formed by ACT ops (rest in one
# wide DVE broadcast op); each ACT chunk costs 4 activation ops
X_ACT_CHUNKS = 0

_DTYPE = "fp8_doublerow"  # informational (test.py prints it)

_CACHE = {}


def _build():
    import concourse.bass as bass
    import concourse.tile as tile
    from concourse import bacc, mybir
    from concourse._compat import axon_active
    from concourse.masks import make_identity

    f32 = mybir.dt.float32
    f32r = mybir.dt.float32r
    bf = mybir.dt.bfloat16
    f8 = mybir.dt.float8e4
    Copy = mybir.ActivationFunctionType.Copy
    DR = mybir.MatmulPerfMode.DoubleRow

    nc = bacc.Bacc(
        "TRN2",
        target_bir_lowering=False,
        debug=not axon_active(),
        num_devices=NCORES,
    )

    mh8_d = nc.dram_tensor("mh8", [N, N], f8, kind="ExternalInput").ap()
    mhb_d = nc.dram_tensor("mhb", [N, N], bf, kind="ExternalInput").ap()
    adjTc_d = nc.dram_tensor("adjTc", [N, ROWS], f32, kind="ExternalInput").ap()
    adjT_d = nc.dram_tensor("adjT", [N, ROWS], f32, kind="ExternalInput").ap()
    mfT_d = nc.dram_tensor("mfT", [N, ROWS], bf, kind="ExternalInput").ap()
    nfT_d = nc.dram_tensor("nfT", [DIN, N], f32, kind="ExternalInput").ap()
    w_d = nc.dram_tensor("w", [DIN, DOUT], f32, kind="ExternalInput").ap()
    out_d = nc.dram_tensor("out", [ROWS, DOUT], f32, kind="ExternalOutput").ap()

    with tile.TileContext(nc) as tc:
        with (
            tc.tile_pool(name="const", bufs=1) as cpool,
            tc.tile_pool(name="x", bufs=3) as xpool,
            tc.tile_pool(name="y", bufs=2) as ypool,
            tc.tile_pool(name="z", bufs=3) as zpool,
            tc.tile_pool(name="py", bufs=2, space="PSUM") as pypool,
            tc.tile_pool(name="pout", bufs=2, space="PSUM") as popool,
        ):
            # ---- resident tiles + input DMA (one multi-dim DMA per tensor;
            # per-DMA overhead dominates transfer time, so merge + order by
            # when compute needs them) ----
            nfT_sb = cpool.tile([P, KC * N], f32, tag="nfT")
            w_sb = cpool.tile([P, KC * DOUT], f32, tag="w")
            mhb_sb = cpool.tile([P, IC * N], bf, tag="mhb")
            mh8_sb = cpool.tile([P, IC * N], f8, tag="mh8")
            adjTc_sb = cpool.tile([P, N], f32, tag="adjTc")
            adjT_sb = cpool.tile([P, N], f32, tag="adjT")
            mfT_sb = cpool.tile([P, N], bf, tag="mfT")
            nc.sync.dma_start(
                w_sb[:].rearrange("p (kc d) -> p kc d", kc=KC),
                w_d.rearrange("(kc p) d -> p kc d", p=P),
            )
            nc.sync.dma_start(
                nfT_sb[:].rearrange("p (kc c d) -> p kc c d", kc=KC, c=N // P),
                nfT_d.rearrange("(kc p) (c d) -> p kc c d", p=P, c=N // P),
            )
            nc.sync.dma_start(
                adjTc_sb[:].rearrange("p (c r) -> p c r", c=N // P),
                adjTc_d.rearrange("(c p) r -> p c r", p=P),
            )
            nc.sync.dma_start(
                mh8_sb[:].rearrange("p (ic j) -> p ic j", ic=IC),
                mh8_d.rearrange("(ic p) j -> p ic j", p=P),
            )
            nc.sync.dma_start(
                mfT_sb[:].rearrange("p (c r) -> p c r", c=N // P),
                mfT_d.rearrange("(c p) r -> p c r", p=P),
            )
            nc.sync.dma_start(
                mhb_sb[:].rearrange("p (ic j) -> p ic j", ic=IC),
                mhb_d.rearrange("(ic p) j -> p ic j", p=P),
            )
            nc.sync.dma_start(
                adjT_sb[:].rearrange("p (c r) -> p c r", c=N // P),
                adjT_d.rearrange("(c p) r -> p c r", p=P),
            )

            # ---- setup compute ----
            # wf[n,d] = sum_k nf[n,k] w[k,d]; chunks of 128 n-rows (f32)
            wf_sb = cpool.tile([P, N], f32, tag="wf")
            for c in range(N // P):
                pt = pypool.tile([P, 1024], f32, tag="py")
                for kc in range(KC):
                    nc.tensor.matmul(
                        pt[:, :DOUT],
                        lhsT=nfT_sb[
                            :, (kc * (N // P) + c) * P : (kc * (N // P) + c + 1) * P
                        ],
                        rhs=w_sb[:, kc * DOUT : (kc + 1) * DOUT],
                        start=(kc == 0),
                        stop=(kc == KC - 1),
                    )
                nc.vector.tensor_copy(wf_sb[:, c * DOUT : (c + 1) * DOUT], pt[:, :DOUT])

            wf_bf = cpool.tile([P, N], bf, tag="wf_bf")
            nc.vector.tensor_copy(wf_bf[:], wf_sb[:])
            # wf4_bf: wf chunk jc replicated 4x along free dim, for Z = Ybf * wf
            wf4_bf = cpool.tile([P, JC * 512], bf, tag="wf4_bf")
            for jc in range(JC):
                for r in range(G4):
                    nc.vector.tensor_copy(
                        wf4_bf[:, jc * 512 + r * DOUT : jc * 512 + (r + 1) * DOUT],
                        wf_bf[:, jc * DOUT : (jc + 1) * DOUT],
                    )

            id_f32 = cpool.tile([P, P], f32, tag="id_f32")
            make_identity(nc, id_f32[:])
            id_bf = cpool.tile([P, P], bf, tag="id_bf")
            nc.vector.tensor_copy(id_bf[:], id_f32[:])

            # Q'[j,d] = sum_i mh'[i,j] wf[i,d]  (bf16)
            Q_sb = cpool.tile([P, N], f32, tag="Q")
            for jc in range(JC):
                pq = pypool.tile([P, 1024], f32, tag="py")
                for ic in range(IC):
                    nc.tensor.matmul(
                        pq[:, :DOUT],
                        lhsT=mhb_sb[:, ic * N + jc * P : ic * N + (jc + 1) * P],
                        rhs=wf_bf[:, ic * DOUT : (ic + 1) * DOUT],
                        start=(ic == 0),
                        stop=(ic == IC - 1),
                    )
                nc.vector.tensor_copy(Q_sb[:, jc * DOUT : (jc + 1) * DOUT], pq[:, :DOUT])
            # Vq = wf .* Q'  (bf16)
            Vq_sb = cpool.tile([P, N], bf, tag="Vq")
            nc.vector.tensor_mul(Vq_sb[:], wf_bf[:], Q_sb[:])

            # S0 = adj @ wf (f32); mwf = mfs @ wf; G = mfs @ Vq
            s0_sb = cpool.tile([P, DOUT], f32, tag="s0")
            mwf_sb = cpool.tile([P, DOUT], f32, tag="mwf")
            g_sb = cpool.tile([P, DOUT], f32, tag="g")
            for dst, lhs_tile, rhs_tile in (
                (s0_sb, adjT_sb, wf_sb),
                (mwf_sb, mfT_sb, wf_bf),
                (g_sb, mfT_sb, Vq_sb),
            ):
                ps = pypool.tile([P, 1024], f32, tag="py")
                for c in range(N // P):
                    nc.tensor.matmul(
                        ps[:, :DOUT],
                        lhsT=lhs_tile[:, c * P : (c + 1) * P],
                        rhs=rhs_tile[:, c * DOUT : (c + 1) * DOUT],
                        start=(c == 0),
                        stop=(c == N // P - 1),
                    )
                nc.vector.tensor_copy(dst[:], ps[:, :DOUT])

            # C = 0.5*(S0 .* mwf + G)   (bf16)
            tmp_sb = cpool.tile([P, DOUT], f32, tag="tmpC")
            C_sb = cpool.tile([P, DOUT], bf, tag="C")
            nc.vector.tensor_mul(tmp_sb[:], s0_sb[:], mwf_sb[:])
            nc.vector.tensor_add(tmp_sb[:], tmp_sb[:], g_sb[:])
            nc.vector.tensor_scalar_mul(C_sb[:], tmp_sb[:], 0.5)

            out_sb = cpool.tile([P, NG * DOUT], f32, tag="out_sb")

            # broadcast views for the X' formation (shared across groups)
            KD = IC - X_ACT_CHUNKS  # chunks covered by the wide DVE op
            wf_bc = (
                wf_sb[:, : KD * DOUT]
                .rearrange("p (ic d) -> p ic d", ic=KD)
                .unsqueeze(2)
                .broadcast_to([P, KD, G4, DOUT])
            )
            adj_r = adjTc_sb[:].rearrange("p (ic r) -> p ic r", ic=IC)

            def emit_matvec(b, z_t):
                # matvec: out[a,d] = sum_j mfs[a,j] Z[j,(s,d)]; 4-way col-tiled
                pout = popool.tile([P, 512], f32, tag="pout")
                for jc in range(JC):
                    for s in range(G4):
                        a = b * G4 + s
                        nc.tensor.matmul(
                            pout[32 * s : 32 * s + 1, :DOUT],
                            lhsT=mfT_sb[:, jc * P + a : jc * P + a + 1],
                            rhs=z_t[:, jc * 512 + s * DOUT : jc * 512 + (s + 1) * DOUT],
                            start=(jc == 0),
                            stop=False,
                            tile_position=(0, 32 * s),
                            skip_group_check=True,
                        )
                # += C[a,:] via one-hot identity column
                for s in range(G4):
                    a = b * G4 + s
                    nc.tensor.matmul(
                        pout[32 * s : 32 * s + 1, :DOUT],
                        lhsT=id_bf[:, a : a + 1],
                        rhs=C_sb[:],
                        start=False,
                        stop=True,
                        tile_position=(0, 32 * s),
                        skip_group_check=True,
                    )
                nc.scalar.activation(
                    out_sb[:, b * DOUT : (b + 1) * DOUT], pout[:, :DOUT], Copy
                )

            # ---- main loop: 32 groups of 4 rows; the matvec of group b is
            # issued after the main matmuls of group b+1 so the PE never
            # stalls on the ACT drain + DVE Z chain ----
            pending_mv = None
            for b in range(NG):
                # X'[i,(s,d)] = adj'[a,i] * wf[i,d]  -> fp8
                x_t = xpool.tile([P, IC * G4 * DOUT], f8, tag="X")
                if KD:
                    adj_bc = (
                        adj_r[:, :KD, G4 * b : G4 * (b + 1)]
                        .unsqueeze(3)
                        .broadcast_to([P, KD, G4, DOUT])
                    )
                    x_v = x_t[:, : KD * G4 * DOUT].rearrange(
                        "p (ic s d) -> p ic s d", ic=KD, s=G4
                    )
                    nc.vector.tensor_mul(x_v, wf_bc, adj_bc)
                for ic in range(KD, IC):
                    for s in range(G4):
                        a = b * G4 + s
                        nc.scalar.activation(
                            x_t[:, ic * 512 + s * DOUT : ic * 512 + (s + 1) * DOUT],
                            wf_sb[:, ic * DOUT : (ic + 1) * DOUT],
                            Copy,
                            scale=adjTc_sb[:, ic * P + a : ic * P + a + 1],
                        )

                # main matmul: Y'[j,(s,d)] accumulated over i-pairs (fp8 DoubleRow)
                ybf = ypool.tile([P, JC * 512], bf, tag="ybf")
                for jh in range(JC // 2):
                    py = pypool.tile([P, 1024], f32, tag="py")
                    for jl in range(2):
                        jc = jh * 2 + jl
                        for icp in range(ICP):
                            lhsT3 = mh8_sb[
                                :, 2 * icp * N : (2 * icp + 2) * N
                            ].rearrange("p (k f) -> p k f", k=2)[
                                :, :, jc * P : (jc + 1) * P
                            ]
                            rhs3 = x_t[
                                :, 2 * icp * 512 : (2 * icp + 2) * 512
                            ].rearrange("p (k f) -> p k f", k=2)
                            nc.tensor.matmul(
                                py[:, jl * 512 : (jl + 1) * 512],
                                lhsT=lhsT3,
                                rhs=rhs3,
                                start=(icp == 0),
                                stop=(icp == ICP - 1),
                                perf_mode=DR,
                            )
                    # drain 2 banks at once on ACT (f32 -> bf16)
                    nc.scalar.activation(
                        ybf[:, jh * 1024 : (jh + 1) * 1024], py[:], Copy
                    )
                if pending_mv is not None:
                    emit_matvec(*pending_mv)
                # Z = Ybf .* wf (bf16, DVE 2x)
                z_t = zpool.tile([P, JC * 512], bf, tag="Z")
                for h in range(2):
                    nc.vector.tensor_mul(
                        z_t[:, h * 2048 : (h + 1) * 2048],
                        ybf[:, h * 2048 : (h + 1) * 2048],
                        wf4_bf[:, h * 2048 : (h + 1) * 2048],
                    )
                pending_mv = (b, z_t)
                if b == NG // 2 + 1:
                    # first half of the output rows is complete -> start DMA
                    for s in range(G4):
                        nc.sync.dma_start(
                            out_d[s : 2 * ROWS // G4 : G4, :],
                            out_sb[32 * s : 32 * s + 1, : NG // 2 * DOUT],
                        )
            emit_matvec(*pending_mv)

            # ---- store: row 4b+s lives at out_sb[32s, b*128:(b+1)*128] ----
            for s in range(G4):
                nc.sync.dma_start(
                    out_d[2 * ROWS // G4 + s :: G4, :],
                    out_sb[32 * s : 32 * s + 1, NG // 2 * DOUT :],
                )

    nc.compile()
    return nc


def _prep_inputs(inputs):
    """Host-side sharding + layout prep. Returns per-core input maps."""
    import ml_dtypes

    bf16 = ml_dtypes.bfloat16
    f8 = ml_dtypes.float8_e4m3

    nf = np.asarray(inputs["node_features"], dtype=np.float32)
    adj = np.asarray(inputs["adjacency_matrix"], dtype=np.float32)
    mf = np.asarray(inputs["mask_father"], dtype=np.float32)[:, 0, :]
    ncnt = np.asarray(inputs["neighbor_count"], dtype=np.float32)
    mh = np.asarray(inputs["mask_hadamard"], dtype=np.float32)[:, 0, :]
    w = np.asarray(inputs["weight"], dtype=np.float32)

    mhp = mh - np.float32(0.5)
    mh8 = mhp.astype(f8)
    mhb = mhp.astype(bf16)
    mfs = mf / (ncnt * ncnt)  # fold 1/ncnt^2 into the father mask
    nfT = np.ascontiguousarray(nf.T)
    in_maps = []
    for c in range(NCORES):
        rows = slice(c * ROWS, (c + 1) * ROWS)
        adjr = adj[rows]
        in_maps.append(
            {
                "mh8": mh8,
                "mhb": mhb,
                "adjTc": np.ascontiguousarray(adjr.T) - np.float32(0.5),
                "adjT": np.ascontiguousarray(adjr.T),
                "mfT": np.ascontiguousarray(mfs[rows].T).astype(bf16),
                "nfT": nfT,
                "w": w,
            }
        )
    return in_maps


def _run(inputs, trace=False):
    from concourse import bass_utils

    if "nc" not in _CACHE:
        _CACHE["nc"] = _build()
    nc = _CACHE["nc"]
    in_maps = _prep_inputs(inputs)
    res = bass_utils.run_bass_kernel_spmd(
        nc, in_maps, core_ids=list(range(NCORES)), trace=trace
    )
    out = np.concatenate([r["out"] for r in res.results], axis=0)
    return out, res


def kernel(**inputs):
    out, _ = _run(inputs, trace=False)
    return out


# revision 21
# speedup vs baseline: 1.0143x; 1.0143x over previous
"""Trainium2 Bass kernel for masked graph-convolution interaction.

Math (reference):
    wf = node_features @ weight                              # [N, D]
    T[i,d,j] = wf[i,d] * wf[j,d] * mh[i,j]
    S[a,d,j] = sum_i adj[a,i] * T[i,d,j]
    out[a,d] = sum_j S[a,d,j] * mf[a,j] / ncnt[a]^2

fp8 scheme (mean-centered so e4m3 quantization error stays ~6e-3):
    mh = 0.5 + mh',  adj = 0.5 + adj',  mfs = mf / ncnt^2 (folded on host)
    X'_a[i,d] = adj'[a,i] * wf[i,d]                 (fp8, DVE/ACT scale-copy)
    Y'_a = mh'8^T @ X'8_a                           (PE, fp8 DoubleRow = 2x)
    out[a,d] = sum_j mfs[a,j] wf[j,d] Y'_a[j,d] + C[a,d]
    C = 0.5*(adj@wf) .* (mfs@wf) + 0.5*mfs@(wf .* (mh'^T@wf))   (exact, tiny)

Stage 2: ACT drains Y' PSUM->SBUF bf16, DVE multiplies by wf (bf16 2x mode),
per-row j-contraction as 4-way col-tiled PE matvecs (lhsT = mfs column), and
the C correction is accumulated into the same PSUM via one-hot id columns.

Sharding: row-split of a across 8 cores (128 rows each); mh / wf replicated.
"""

import numpy as np

N = 1024
DIN = 256
DOUT = 128
NCORES = 8
ROWS = N // NCORES  # 128 output rows per core
P = 128
IC = N // P  # 8 chunks over i
ICP = IC // 2  # 4 DoubleRow pairs
JC = N // P  # 8 chunks over j
KC = DIN // P  # 2 chunks over k (wf compute)
G4 = 4  # rows per group
NG = ROWS // G4  # 32 groups per core

# how many of the 8 per-group X' ic-chunks formed by ACT ops (rest in one
# wide DVE broadcast op); each ACT chunk costs 4 activation ops
X_ACT_CHUNKS = 0

_DTYPE = "fp8_doublerow"  # informational (test.py prints it)

_CACHE = {}


def _build():
    import concourse.bass as bass
    import concourse.tile as tile
    from concourse import bacc, mybir
    from concourse._compat import axon_active
    from concourse.masks import make_identity

    f32 = mybir.dt.float32
    f32r = mybir.dt.float32r
    bf = mybir.dt.bfloat16
    f8 = mybir.dt.float8e4
    Copy = mybir.ActivationFunctionType.Copy
    DR = mybir.MatmulPerfMode.DoubleRow

    nc = bacc.Bacc(
        "TRN2",
        target_bir_lowering=False,
        debug=not axon_active(),
        num_devices=NCORES,
    )

    mh8_d = nc.dram_tensor("mh8", [N, N], f8, kind="ExternalInput").ap()
    mhb_d = nc.dram_tensor("mhb", [N, N], bf, kind="ExternalInput").ap()
    adjTc_d = nc.dram_tensor("adjTc", [N, ROWS], f32, kind="ExternalInput").ap()
    adjT_d = nc.dram_tensor("adjT", [N, ROWS], f32, kind="ExternalInput").ap()
    mfT_d = nc.dram_tensor("mfT", [N, ROWS], bf, kind="ExternalInput").ap()
    nfT_d = nc.dram_tensor("nfT", [DIN, N], f32, kind="ExternalInput").ap()
    w_d = nc.dram_tensor("w", [DIN, DOUT], f32, kind="ExternalInput").ap()
    out_d = nc.dram_tensor("out", [ROWS, DOUT], f32, kind="ExternalOutput").ap()

    with tile.TileContext(nc) as tc:
        with (
            tc.tile_pool(name="const", bufs=1) as cpool,
            tc.tile_pool(name="x", bufs=3) as xpool,
            tc.tile_pool(name="y", bufs=2) as ypool,
            tc.tile_pool(name="z", bufs=3) as zpool,
            tc.tile_pool(name="py", bufs=2, space="PSUM") as pypool,
            tc.tile_pool(name="pout", bufs=2, space="PSUM") as popool,
        ):
            # ---- resident tiles + input DMA (one multi-dim DMA per tensor;
            # per-DMA overhead dominates transfer time, so merge + order by
            # when compute needs them) ----
            nfT_sb = cpool.tile([P, KC * N], f32, tag="nfT")
            w_sb = cpool.tile([P, KC * DOUT], f32, tag="w")
            mhb_sb = cpool.tile([P, IC * N], bf, tag="mhb")
            mh8_sb = cpool.tile([P, IC * N], f8, tag="mh8")
            adjTc_sb = cpool.tile([P, N], f32, tag="adjTc")
            adjT_sb = cpool.tile([P, N], f32, tag="adjT")
            mfT_sb = cpool.tile([P, N], bf, tag="mfT")
            nc.sync.dma_start(
                w_sb[:].rearrange("p (kc d) -> p kc d", kc=KC),
                w_d.rearrange("(kc p) d -> p kc d", p=P),
            )
            nc.sync.dma_start(
                nfT_sb[:].rearrange("p (kc c d) -> p kc c d", kc=KC, c=N // P),
                nfT_d.rearrange("(kc p) (c d) -> p kc c d", p=P, c=N // P),
            )
            nc.sync.dma_start(
                adjTc_sb[:].rearrange("p (c r) -> p c r", c=N // P),
                adjTc_d.rearrange("(c p) r -> p c r", p=P),
            )
            nc.sync.dma_start(
                mh8_sb[:].rearrange("p (ic j) -> p ic j", ic=IC),
                mh8_d.rearrange("(ic p) j -> p ic j", p=P),
            )
            nc.sync.dma_start(
                mfT_sb[:].rearrange("p (c r) -> p c r", c=N // P),
                mfT_d.rearrange("(c p) r -> p c r", p=P),
            )
            nc.sync.dma_start(
                mhb_sb[:].rearrange("p (ic j) -> p ic j", ic=IC),
                mhb_d.rearrange("(ic p) j -> p ic j", p=P),
            )
            nc.sync.dma_start(
                adjT_sb[:].rearrange("p (c r) -> p c r", c=N // P),
                adjT_d.rearrange("(c p) r -> p c r", p=P),
            )

            # ---- setup compute ----
            # wf[n,d] = sum_k nf[n,k] w[k,d]; chunks of 128 n-rows (f32)
            wf_sb = cpool.tile([P, N], f32, tag="wf")
            for c in range(N // P):
                pt = pypool.tile([P, 1024], f32, tag="py")
                for kc in range(KC):
                    nc.tensor.matmul(
                        pt[:, :DOUT],
                        lhsT=nfT_sb[
                            :, (kc * (N // P) + c) * P : (kc * (N // P) + c + 1) * P
                        ],
                        rhs=w_sb[:, kc * DOUT : (kc + 1) * DOUT],
                        start=(kc == 0),
                        stop=(kc == KC - 1),
                    )
                nc.vector.tensor_copy(wf_sb[:, c * DOUT : (c + 1) * DOUT], pt[:, :DOUT])

            wf_bf = cpool.tile([P, N], bf, tag="wf_bf")
            nc.vector.tensor_copy(wf_bf[:], wf_sb[:])
            # wf4_bf: wf chunk jc replicated 4x along free dim, for Z = Ybf * wf
            wf4_bf = cpool.tile([P, JC * 512], bf, tag="wf4_bf")
            for jc in range(JC):
                for r in range(G4):
                    nc.vector.tensor_copy(
                        wf4_bf[:, jc * 512 + r * DOUT : jc * 512 + (r + 1) * DOUT],
                        wf_bf[:, jc * DOUT : (jc + 1) * DOUT],
                    )

            id_f32 = cpool.tile([P, P], f32, tag="id_f32")
            make_identity(nc, id_f32[:])
            id_bf = cpool.tile([P, P], bf, tag="id_bf")
            nc.vector.tensor_copy(id_bf[:], id_f32[:])

            # corrections: emitted after group 0's main matmuls so the PE
            # isn't stalled on the late mhb/adjT DMAs; C is first consumed
            # by matvec(0), which runs after main(1).
            C_sb = cpool.tile([P, DOUT], bf, tag="C")

            def emit_corrections():
                # Q'[j,d] = sum_i mh'[i,j] wf[i,d]  (bf16)
                Q_sb = cpool.tile([P, N], f32, tag="Q")
                for jc in range(JC):
                    pq = pypool.tile([P, 1024], f32, tag="py")
                    for ic in range(IC):
                        nc.tensor.matmul(
                            pq[:, :DOUT],
                            lhsT=mhb_sb[:, ic * N + jc * P : ic * N + (jc + 1) * P],
                            rhs=wf_bf[:, ic * DOUT : (ic + 1) * DOUT],
                            start=(ic == 0),
                            stop=(ic == IC - 1),
                        )
                    nc.vector.tensor_copy(
                        Q_sb[:, jc * DOUT : (jc + 1) * DOUT], pq[:, :DOUT]
                    )
                # Vq = wf .* Q'  (bf16)
                Vq_sb = cpool.tile([P, N], bf, tag="Vq")
                nc.vector.tensor_mul(Vq_sb[:], wf_bf[:], Q_sb[:])

                # S0 = adj @ wf (f32); mwf = mfs @ wf; G = mfs @ Vq
                s0_sb = cpool.tile([P, DOUT], f32, tag="s0")
                mwf_sb = cpool.tile([P, DOUT], f32, tag="mwf")
                g_sb = cpool.tile([P, DOUT], f32, tag="g")
                for dst, lhs_tile, rhs_tile in (
                    (s0_sb, adjT_sb, wf_sb),
                    (mwf_sb, mfT_sb, wf_bf),
                    (g_sb, mfT_sb, Vq_sb),
                ):
                    ps = pypool.tile([P, 1024], f32, tag="py")
                    for c in range(N // P):
                        nc.tensor.matmul(
                            ps[:, :DOUT],
                            lhsT=lhs_tile[:, c * P : (c + 1) * P],
                            rhs=rhs_tile[:, c * DOUT : (c + 1) * DOUT],
                            start=(c == 0),
                            stop=(c == N // P - 1),
                        )
                    nc.vector.tensor_copy(dst[:], ps[:, :DOUT])

                # C = 0.5*(S0 .* mwf + G)   (bf16)
                tmp_sb = cpool.tile([P, DOUT], f32, tag="tmpC")
                nc.vector.tensor_mul(tmp_sb[:], s0_sb[:], mwf_sb[:])
                nc.vector.tensor_add(tmp_sb[:], tmp_sb[:], g_sb[:])
                nc.vector.tensor_scalar_mul(C_sb[:], tmp_sb[:], 0.5)

            out_sb = cpool.tile([P, NG * DOUT], f32, tag="out_sb")

            # broadcast views for the X' formation (shared across groups)
            KD = IC - X_ACT_CHUNKS  # chunks covered by the wide DVE op
            wf_bc = (
                wf_sb[:, : KD * DOUT]
                .rearrange("p (ic d) -> p ic d", ic=KD)
                .unsqueeze(2)
                .broadcast_to([P, KD, G4, DOUT])
            )
            adj_r = adjTc_sb[:].rearrange("p (ic r) -> p ic r", ic=IC)

            def emit_matvec(b, z_t):
                # matvec: out[a,d] = sum_j mfs[a,j] Z[j,(s,d)]; 4-way col-tiled
                pout = popool.tile([P, 512], f32, tag="pout")
                for jc in range(JC):
                    for s in range(G4):
                        a = b * G4 + s
                        nc.tensor.matmul(
                            pout[32 * s : 32 * s + 1, :DOUT],
                            lhsT=mfT_sb[:, jc * P + a : jc * P + a + 1],
                            rhs=z_t[:, jc * 512 + s * DOUT : jc * 512 + (s + 1) * DOUT],
                            start=(jc == 0),
                            stop=False,
                            tile_position=(0, 32 * s),
                            skip_group_check=True,
                        )
                # += C[a,:] via one-hot identity column
                for s in range(G4):
                    a = b * G4 + s
                    nc.tensor.matmul(
                        pout[32 * s : 32 * s + 1, :DOUT],
                        lhsT=id_bf[:, a : a + 1],
                        rhs=C_sb[:],
                        start=False,
                        stop=True,
                        tile_position=(0, 32 * s),
                        skip_group_check=True,
                    )
                nc.scalar.activation(
                    out_sb[:, b * DOUT : (b + 1) * DOUT], pout[:, :DOUT], Copy
                )

            # ---- main loop: 32 groups of 4 rows; the matvec of group b is
            # issued after the main matmuls of group b+1 so the PE never
            # stalls on the ACT drain + DVE Z chain ----
            pending_mv = None
            for b in range(NG):
                # X'[i,(s,d)] = adj'[a,i] * wf[i,d]  -> fp8
                x_t = xpool.tile([P, IC * G4 * DOUT], f8, tag="X")
                if KD:
                    adj_bc = (
                        adj_r[:, :KD, G4 * b : G4 * (b + 1)]
                        .unsqueeze(3)
                        .broadcast_to([P, KD, G4, DOUT])
                    )
                    x_v = x_t[:, : KD * G4 * DOUT].rearrange(
                        "p (ic s d) -> p ic s d", ic=KD, s=G4
                    )
                    nc.vector.tensor_mul(x_v, wf_bc, adj_bc)
                for ic in range(KD, IC):
                    for s in range(G4):
                        a = b * G4 + s
                        nc.scalar.activation(
                            x_t[:, ic * 512 + s * DOUT : ic * 512 + (s + 1) * DOUT],
                            wf_sb[:, ic * DOUT : (ic + 1) * DOUT],
                            Copy,
                            scale=adjTc_sb[:, ic * P + a : ic * P + a + 1],
                        )

                # main matmul: Y'[j,(s,d)] accumulated over i-pairs (fp8 DoubleRow)
                ybf = ypool.tile([P, JC * 512], bf, tag="ybf")
                for jh in range(JC // 2):
                    py = pypool.tile([P, 1024], f32, tag="py")
                    for jl in range(2):
                        jc = jh * 2 + jl
                        for icp in range(ICP):
                            lhsT3 = mh8_sb[
                                :, 2 * icp * N : (2 * icp + 2) * N
                            ].rearrange("p (k f) -> p k f", k=2)[
                                :, :, jc * P : (jc + 1) * P
                            ]
                            rhs3 = x_t[
                                :, 2 * icp * 512 : (2 * icp + 2) * 512
                            ].rearrange("p (k f) -> p k f", k=2)
                            nc.tensor.matmul(
                                py[:, jl * 512 : (jl + 1) * 512],
                                lhsT=lhsT3,
                                rhs=rhs3,
                                start=(icp == 0),
                                stop=(icp == ICP - 1),
                                perf_mode=DR,
                            )
                    # drain 2 banks at once on ACT (f32 -> bf16)
                    nc.scalar.activation(
                        ybf[:, jh * 1024 : (jh + 1) * 1024], py[:], Copy
                    )
                if b == 0:
                    emit_corrections()
                if pending_mv is not None:
                    emit_matvec(*pending_mv)
                # Z = Ybf .* wf (bf16, DVE 2x)
                z_t = zpool.tile([P, JC * 512], bf, tag="Z")
                for h in range(2):
                    nc.vector.tensor_mul(
                        z_t[:, h * 2048 : (h + 1) * 2048],
                        ybf[:, h * 2048 : (h + 1) * 2048],
                        wf4_bf[:, h * 2048 : (h + 1) * 2048],
                    )
                pending_mv = (b, z_t)
                if b == NG // 2 + 1:
                    # first half of the output rows is complete -> start DMA
                    for s in range(G4):
                        nc.sync.dma_start(
                            out_d[s : 2 * ROWS // G4 : G4, :],
                            out_sb[32 * s : 32 * s + 1, : NG // 2 * DOUT],
                        )
            emit_matvec(*pending_mv)

            # ---- store: row 4b+s lives at out_sb[32s, b*128:(b+1)*128] ----
            for s in range(G4):
                nc.sync.dma_start(
                    out_d[2 * ROWS // G4 + s :: G4, :],
                    out_sb[32 * s : 32 * s + 1, NG // 2 * DOUT :],
                )

    nc.compile()
    return nc


def _prep_inputs(inputs):
    """Host-side sharding + layout prep. Returns per-core input maps."""
    import ml_dtypes

    bf16 = ml_dtypes.bfloat16
    f8 = ml_dtypes.float8_e4m3

    nf = np.asarray(inputs["node_features"], dtype=np.float32)
    adj = np.asarray(inputs["adjacency_matrix"], dtype=np.float32)
    mf = np.asarray(inputs["mask_father"], dtype=np.float32)[:, 0, :]
    ncnt = np.asarray(inputs["neighbor_count"], dtype=np.float32)
    mh = np.asarray(inputs["mask_hadamard"], dtype=np.float32)[:, 0, :]
    w = np.asarray(inputs["weight"], dtype=np.float32)

    mhp = mh - np.float32(0.5)
    mh8 = mhp.astype(f8)
    mhb = mhp.astype(bf16)
    mfs = mf / (ncnt * ncnt)  # fold 1/ncnt^2 into the father mask
    nfT = np.ascontiguousarray(nf.T)
    in_maps = []
    for c in range(NCORES):
        rows = slice(c * ROWS, (c + 1) * ROWS)
        adjr = adj[rows]
        in_maps.append(
            {
                "mh8": mh8,
                "mhb": mhb,
                "adjTc": np.ascontiguousarray(adjr.T) - np.float32(0.5),
                "adjT": np.ascontiguousarray(adjr.T),
                "mfT": np.ascontiguousarray(mfs[rows].T).astype(bf16),
                "nfT": nfT,
                "w": w,
            }
        )
    return in_maps


def _run(inputs, trace=False):
    from concourse import bass_utils

    if "nc" not in _CACHE:
        _CACHE["nc"] = _build()
    nc = _CACHE["nc"]
    in_maps = _prep_inputs(inputs)
    res = bass_utils.run_bass_kernel_spmd(
        nc, in_maps, core_ids=list(range(NCORES)), trace=trace
    )
    out = np.concatenate([r["out"] for r in res.results], axis=0)
    return out, res


def kernel(**inputs):
    out, _ = _run(inputs, trace=False)
    return out


# revision 23
# speedup vs baseline: 1.0884x; 1.0731x over previous
"""Trainium2 Bass kernel for masked graph-convolution interaction.

Math (reference):
    wf = node_features @ weight                              # [N, D]
    T[i,d,j] = wf[i,d] * wf[j,d] * mh[i,j]
    S[a,d,j] = sum_i adj[a,i] * T[i,d,j]
    out[a,d] = sum_j S[a,d,j] * mf[a,j] / ncnt[a]^2

fp8 scheme (mean-centered so e4m3 quantization error stays ~6e-3):
    mh = 0.5 + mh',  adj = 0.5 + adj',  mfs = mf / ncnt^2 (folded on host)
    X'_a[i,d] = adj'[a,i] * wf[i,d]                 (fp8, DVE/ACT scale-copy)
    Y'_a = mh'8^T @ X'8_a                           (PE, fp8 DoubleRow = 2x)
    out[a,d] = sum_j mfs[a,j] wf[j,d] Y'_a[j,d] + C[a,d]
    C = 0.5*(adj@wf) .* (mfs@wf) + 0.5*mfs@(wf .* (mh'^T@wf))   (exact, tiny)

Stage 2: ACT drains Y' PSUM->SBUF bf16, DVE multiplies by wf (bf16 2x mode),
per-row j-contraction as 4-way col-tiled PE matvecs (lhsT = mfs column), and
the C correction is accumulated into the same PSUM via one-hot id columns.

Sharding: row-split of a across 8 cores (128 rows each); mh / wf replicated.
"""

import numpy as np

N = 1024
DIN = 256
DOUT = 128
NCORES = 8
ROWS = N // NCORES  # 128 output rows per core
P = 128
IC = N // P  # 8 chunks over i
ICP = IC // 2  # 4 DoubleRow pairs
JC = N // P  # 8 chunks over j
KC = DIN // P  # 2 chunks over k (wf compute)
G4 = 4  # rows per group
NG = ROWS // G4  # 32 groups per core

# how many of the 8 per-group X' ic-chunks formed by ACT ops (rest in one
# wide DVE broadcast op); each ACT chunk costs 4 activation ops
X_ACT_CHUNKS = 0

_DTYPE = "fp8_doublerow"  # informational (test.py prints it)

_CACHE = {}


def _build():
    import concourse.bass as bass
    import concourse.tile as tile
    from concourse import bacc, mybir
    from concourse._compat import axon_active
    from concourse.masks import make_identity

    f32 = mybir.dt.float32
    f32r = mybir.dt.float32r
    bf = mybir.dt.bfloat16
    f8 = mybir.dt.float8e4
    Copy = mybir.ActivationFunctionType.Copy
    DR = mybir.MatmulPerfMode.DoubleRow

    nc = bacc.Bacc(
        "TRN2",
        target_bir_lowering=False,
        debug=not axon_active(),
        num_devices=NCORES,
    )

    mh8_d = nc.dram_tensor("mh8", [N, N], f8, kind="ExternalInput").ap()
    mhb_d = nc.dram_tensor("mhb", [N, N], bf, kind="ExternalInput").ap()
    adjTc_d = nc.dram_tensor("adjTc", [N, ROWS], f32, kind="ExternalInput").ap()
    adjT_d = nc.dram_tensor("adjT", [N, ROWS], f32, kind="ExternalInput").ap()
    mfT_d = nc.dram_tensor("mfT", [N, ROWS], bf, kind="ExternalInput").ap()
    nfT_d = nc.dram_tensor("nfT", [DIN, N], f32, kind="ExternalInput").ap()
    w_d = nc.dram_tensor("w", [DIN, DOUT], f32, kind="ExternalInput").ap()
    out_d = nc.dram_tensor("out", [ROWS, DOUT], f32, kind="ExternalOutput").ap()

    with tile.TileContext(nc) as tc:
        with (
            tc.tile_pool(name="const", bufs=1) as cpool,
            tc.tile_pool(name="x", bufs=3) as xpool,
            tc.tile_pool(name="y", bufs=2) as ypool,
            tc.tile_pool(name="z", bufs=3) as zpool,
            tc.tile_pool(name="py", bufs=3, space="PSUM") as pypool,
            tc.tile_pool(name="pout", bufs=2, space="PSUM") as popool,
        ):
            # ---- resident tiles + input DMA (one multi-dim DMA per tensor;
            # per-DMA overhead dominates transfer time, so merge + order by
            # when compute needs them) ----
            nfT_sb = cpool.tile([P, KC * N], f32, tag="nfT")
            w_sb = cpool.tile([P, KC * DOUT], f32, tag="w")
            mhb_sb = cpool.tile([P, IC * N], bf, tag="mhb")
            mh8_sb = cpool.tile([P, IC * N], f8, tag="mh8")
            adjTc_sb = cpool.tile([P, N], f32, tag="adjTc")
            adjT_sb = cpool.tile([P, N], f32, tag="adjT")
            mfT_sb = cpool.tile([P, N], bf, tag="mfT")
            nc.sync.dma_start(
                w_sb[:].rearrange("p (kc d) -> p kc d", kc=KC),
                w_d.rearrange("(kc p) d -> p kc d", p=P),
            )
            nfT_v = nfT_sb[:].rearrange("p (kc c d) -> p kc c d", kc=KC, c=N // P)
            nfT_s = nfT_d.rearrange("(kc p) (c d) -> p kc c d", p=P, c=N // P)
            nc.sync.dma_start(nfT_v[:, :, : N // P // 2], nfT_s[:, :, : N // P // 2])
            nc.sync.dma_start(nfT_v[:, :, N // P // 2 :], nfT_s[:, :, N // P // 2 :])
            nc.sync.dma_start(
                adjTc_sb[:].rearrange("p (c r) -> p c r", c=N // P),
                adjTc_d.rearrange("(c p) r -> p c r", p=P),
            )
            nc.sync.dma_start(
                mh8_sb[:].rearrange("p (ic j) -> p ic j", ic=IC),
                mh8_d.rearrange("(ic p) j -> p ic j", p=P),
            )
            nc.sync.dma_start(
                mfT_sb[:].rearrange("p (c r) -> p c r", c=N // P),
                mfT_d.rearrange("(c p) r -> p c r", p=P),
            )
            nc.sync.dma_start(
                mhb_sb[:].rearrange("p (ic j) -> p ic j", ic=IC),
                mhb_d.rearrange("(ic p) j -> p ic j", p=P),
            )
            nc.sync.dma_start(
                adjT_sb[:].rearrange("p (c r) -> p c r", c=N // P),
                adjT_d.rearrange("(c p) r -> p c r", p=P),
            )

            # ---- setup compute ----
            # wf[n,d] = sum_k nf[n,k] w[k,d]; chunks of 128 n-rows (f32)
            wf_sb = cpool.tile([P, N], f32, tag="wf")
            for c in range(N // P):
                pt = pypool.tile([P, 1024], f32, tag="py")
                for kc in range(KC):
                    nc.tensor.matmul(
                        pt[:, :DOUT],
                        lhsT=nfT_sb[
                            :, (kc * (N // P) + c) * P : (kc * (N // P) + c + 1) * P
                        ],
                        rhs=w_sb[:, kc * DOUT : (kc + 1) * DOUT],
                        start=(kc == 0),
                        stop=(kc == KC - 1),
                    )
                nc.vector.tensor_copy(wf_sb[:, c * DOUT : (c + 1) * DOUT], pt[:, :DOUT])

            wf_bf = cpool.tile([P, N], bf, tag="wf_bf")
            nc.vector.tensor_copy(wf_bf[:], wf_sb[:])
            # wf4_bf: wf chunk jc replicated 4x along free dim, for Z = Ybf * wf
            wf4_bf = cpool.tile([P, JC * 512], bf, tag="wf4_bf")
            for jc in range(JC):
                for r in range(G4):
                    nc.vector.tensor_copy(
                        wf4_bf[:, jc * 512 + r * DOUT : jc * 512 + (r + 1) * DOUT],
                        wf_bf[:, jc * DOUT : (jc + 1) * DOUT],
                    )

            id_f32 = cpool.tile([P, P], f32, tag="id_f32")
            make_identity(nc, id_f32[:])
            id_bf = cpool.tile([P, P], bf, tag="id_bf")
            nc.vector.tensor_copy(id_bf[:], id_f32[:])

            # corrections: emitted after group 0's main matmuls so the PE
            # isn't stalled on the late mhb/adjT DMAs; C is first consumed
            # by matvec(0), which runs after main(1).
            C_sb = cpool.tile([P, DOUT], bf, tag="C")

            def emit_corrections():
                # Q'[j,d] = sum_i mh'[i,j] wf[i,d]  (bf16)
                Q_sb = cpool.tile([P, N], f32, tag="Q")
                for jc in range(JC):
                    pq = pypool.tile([P, 1024], f32, tag="py")
                    for ic in range(IC):
                        nc.tensor.matmul(
                            pq[:, :DOUT],
                            lhsT=mhb_sb[:, ic * N + jc * P : ic * N + (jc + 1) * P],
                            rhs=wf_bf[:, ic * DOUT : (ic + 1) * DOUT],
                            start=(ic == 0),
                            stop=(ic == IC - 1),
                        )
                    nc.vector.tensor_copy(
                        Q_sb[:, jc * DOUT : (jc + 1) * DOUT], pq[:, :DOUT]
                    )
                # Vq = wf .* Q'  (bf16)
                Vq_sb = cpool.tile([P, N], bf, tag="Vq")
                nc.vector.tensor_mul(Vq_sb[:], wf_bf[:], Q_sb[:])

                # S0 = adj @ wf (f32); mwf = mfs @ wf; G = mfs @ Vq
                s0_sb = cpool.tile([P, DOUT], f32, tag="s0")
                mwf_sb = cpool.tile([P, DOUT], f32, tag="mwf")
                g_sb = cpool.tile([P, DOUT], f32, tag="g")
                for dst, lhs_tile, rhs_tile in (
                    (s0_sb, adjT_sb, wf_sb),
                    (mwf_sb, mfT_sb, wf_bf),
                    (g_sb, mfT_sb, Vq_sb),
                ):
                    ps = pypool.tile([P, 1024], f32, tag="py")
                    for c in range(N // P):
                        nc.tensor.matmul(
                            ps[:, :DOUT],
                            lhsT=lhs_tile[:, c * P : (c + 1) * P],
                            rhs=rhs_tile[:, c * DOUT : (c + 1) * DOUT],
                            start=(c == 0),
                            stop=(c == N // P - 1),
                        )
                    nc.vector.tensor_copy(dst[:], ps[:, :DOUT])

                # C = 0.5*(S0 .* mwf + G)   (bf16)
                tmp_sb = cpool.tile([P, DOUT], f32, tag="tmpC")
                nc.vector.tensor_mul(tmp_sb[:], s0_sb[:], mwf_sb[:])
                nc.vector.tensor_add(tmp_sb[:], tmp_sb[:], g_sb[:])
                nc.vector.tensor_scalar_mul(C_sb[:], tmp_sb[:], 0.5)

            out_sb = cpool.tile([P, NG * DOUT], f32, tag="out_sb")

            # broadcast views for the X' formation (shared across groups)
            KD = IC - X_ACT_CHUNKS  # chunks covered by the wide DVE op
            wf_bc = (
                wf_sb[:, : KD * DOUT]
                .rearrange("p (ic d) -> p ic d", ic=KD)
                .unsqueeze(2)
                .broadcast_to([P, KD, G4, DOUT])
            )
            adj_r = adjTc_sb[:].rearrange("p (ic r) -> p ic r", ic=IC)

            def emit_matvec(b, z_t):
                # matvec: out[a,d] = sum_j mfs[a,j] Z[j,(s,d)]; 4-way col-tiled
                pout = popool.tile([P, 512], f32, tag="pout")
                for jc in range(JC):
                    for s in range(G4):
                        a = b * G4 + s
                        nc.tensor.matmul(
                            pout[32 * s : 32 * s + 1, :DOUT],
                            lhsT=mfT_sb[:, jc * P + a : jc * P + a + 1],
                            rhs=z_t[:, jc * 512 + s * DOUT : jc * 512 + (s + 1) * DOUT],
                            start=(jc == 0),
                            stop=False,
                            tile_position=(0, 32 * s),
                            skip_group_check=True,
                        )
                # += C[a,:] via one-hot identity column
                for s in range(G4):
                    a = b * G4 + s
                    nc.tensor.matmul(
                        pout[32 * s : 32 * s + 1, :DOUT],
                        lhsT=id_bf[:, a : a + 1],
                        rhs=C_sb[:],
                        start=False,
                        stop=True,
                        tile_position=(0, 32 * s),
                        skip_group_check=True,
                    )
                nc.scalar.activation(
                    out_sb[:, b * DOUT : (b + 1) * DOUT], pout[:, :DOUT], Copy
                )

            # ---- main loop: 32 groups of 4 rows; the matvec of group b is
            # issued after the main matmuls of group b+1 so the PE never
            # stalls on the ACT drain + DVE Z chain ----
            pending_mv = None
            for b in range(NG):
                # X'[i,(s,d)] = adj'[a,i] * wf[i,d]  -> fp8
                x_t = xpool.tile([P, IC * G4 * DOUT], f8, tag="X")
                if KD:
                    adj_bc = (
                        adj_r[:, :KD, G4 * b : G4 * (b + 1)]
                        .unsqueeze(3)
                        .broadcast_to([P, KD, G4, DOUT])
                    )
                    x_v = x_t[:, : KD * G4 * DOUT].rearrange(
                        "p (ic s d) -> p ic s d", ic=KD, s=G4
                    )
                    nc.vector.tensor_mul(x_v, wf_bc, adj_bc)
                for ic in range(KD, IC):
                    for s in range(G4):
                        a = b * G4 + s
                        nc.scalar.activation(
                            x_t[:, ic * 512 + s * DOUT : ic * 512 + (s + 1) * DOUT],
                            wf_sb[:, ic * DOUT : (ic + 1) * DOUT],
                            Copy,
                            scale=adjTc_sb[:, ic * P + a : ic * P + a + 1],
                        )

                # main matmul: Y'[j,(s,d)] accumulated over i-pairs (fp8 DoubleRow)
                ybf = ypool.tile([P, JC * 512], bf, tag="ybf")
                for jh in range(JC // 2):
                    py = pypool.tile([P, 1024], f32, tag="py")
                    for jl in range(2):
                        jc = jh * 2 + jl
                        for icp in range(ICP):
                            lhsT3 = mh8_sb[
                                :, 2 * icp * N : (2 * icp + 2) * N
                            ].rearrange("p (k f) -> p k f", k=2)[
                                :, :, jc * P : (jc + 1) * P
                            ]
                            rhs3 = x_t[
                                :, 2 * icp * 512 : (2 * icp + 2) * 512
                            ].rearrange("p (k f) -> p k f", k=2)
                            nc.tensor.matmul(
                                py[:, jl * 512 : (jl + 1) * 512],
                                lhsT=lhsT3,
                                rhs=rhs3,
                                start=(icp == 0),
                                stop=(icp == ICP - 1),
                                perf_mode=DR,
                            )
                    # drain 2 banks at once on ACT (f32 -> bf16)
                    nc.scalar.activation(
                        ybf[:, jh * 1024 : (jh + 1) * 1024], py[:], Copy
                    )
                if b == 0:
                    emit_corrections()
                if pending_mv is not None:
                    emit_matvec(*pending_mv)
                # Z = Ybf .* wf (bf16, DVE 2x)
                z_t = zpool.tile([P, JC * 512], bf, tag="Z")
                for h in range(2):
                    nc.vector.tensor_mul(
                        z_t[:, h * 2048 : (h + 1) * 2048],
                        ybf[:, h * 2048 : (h + 1) * 2048],
                        wf4_bf[:, h * 2048 : (h + 1) * 2048],
                    )
                pending_mv = (b, z_t)
                if b == NG // 2 + 1:
                    # first half of the output rows is complete -> start DMA
                    for s in range(G4):
                        nc.sync.dma_start(
                            out_d[s : 2 * ROWS // G4 : G4, :],
                            out_sb[32 * s : 32 * s + 1, : NG // 2 * DOUT],
                        )
            emit_matvec(*pending_mv)

            # ---- store: row 4b+s lives at out_sb[32s, b*128:(b+1)*128] ----
            for s in range(G4):
                nc.sync.dma_start(
                    out_d[2 * ROWS // G4 + s :: G4, :],
                    out_sb[32 * s : 32 * s + 1, NG // 2 * DOUT :],
                )

    nc.compile()
    return nc


def _prep_inputs(inputs):
    """Host-side sharding + layout prep. Returns per-core input maps."""
    import ml_dtypes

    bf16 = ml_dtypes.bfloat16
    f8 = ml_dtypes.float8_e4m3

    nf = np.asarray(inputs["node_features"], dtype=np.float32)
    adj = np.asarray(inputs["adjacency_matrix"], dtype=np.float32)
    mf = np.asarray(inputs["mask_father"], dtype=np.float32)[:, 0, :]
    ncnt = np.asarray(inputs["neighbor_count"], dtype=np.float32)
    mh = np.asarray(inputs["mask_hadamard"], dtype=np.float32)[:, 0, :]
    w = np.asarray(inputs["weight"], dtype=np.float32)

    mhp = mh - np.float32(0.5)
    mh8 = mhp.astype(f8)
    mhb = mhp.astype(bf16)
    mfs = mf / (ncnt * ncnt)  # fold 1/ncnt^2 into the father mask
    nfT = np.ascontiguousarray(nf.T)
    in_maps = []
    for c in range(NCORES):
        rows = slice(c * ROWS, (c + 1) * ROWS)
        adjr = adj[rows]
        in_maps.append(
            {
                "mh8": mh8,
                "mhb": mhb,
                "adjTc": np.ascontiguousarray(adjr.T) - np.float32(0.5),
                "adjT": np.ascontiguousarray(adjr.T),
                "mfT": np.ascontiguousarray(mfs[rows].T).astype(bf16),
                "nfT": nfT,
                "w": w,
            }
        )
    return in_maps


def _run(inputs, trace=False):
    from concourse import bass_utils

    if "nc" not in _CACHE:
        _CACHE["nc"] = _build()
    nc = _CACHE["nc"]
    in_maps = _prep_inputs(inputs)
    res = bass_utils.run_bass_kernel_spmd(
        nc, in_maps, core_ids=list(range(NCORES)), trace=trace
    )
    out = np.concatenate([r["out"] for r in res.results], axis=0)
    return out, res


def kernel(**inputs):
    out, _ = _run(inputs, trace=False)
    return out


# revision 25
# speedup vs baseline: 1.0963x; 1.0072x over previous
"""Trainium2 Bass kernel for masked graph-convolution interaction.

Math (reference):
    wf = node_features @ weight                              # [N, D]
    T[i,d,j] = wf[i,d] * wf[j,d] * mh[i,j]
    S[a,d,j] = sum_i adj[a,i] * T[i,d,j]
    out[a,d] = sum_j S[a,d,j] * mf[a,j] / ncnt[a]^2

fp8 scheme (mean-centered so e4m3 quantization error stays ~6e-3):
    mh = 0.5 + mh',  adj = 0.5 + adj',  mfs = mf / ncnt^2 (folded on host)
    X'_a[i,d] = adj'[a,i] * wf[i,d]                 (fp8, DVE/ACT scale-copy)
    Y'_a = mh'8^T @ X'8_a                           (PE, fp8 DoubleRow = 2x)
    out[a,d] = sum_j mfs[a,j] wf[j,d] Y'_a[j,d] + C[a,d]
    C = 0.5*(adj@wf) .* (mfs@wf) + 0.5*mfs@(wf .* (mh'^T@wf))   (exact, tiny)

Stage 2: ACT drains Y' PSUM->SBUF bf16, DVE multiplies by wf (bf16 2x mode),
per-row j-contraction as 4-way col-tiled PE matvecs (lhsT = mfs column), and
the C correction is accumulated into the same PSUM via one-hot id columns.

Sharding: row-split of a across 8 cores (128 rows each); mh / wf replicated.
"""

import numpy as np

N = 1024
DIN = 256
DOUT = 128
NCORES = 8
ROWS = N // NCORES  # 128 output rows per core
P = 128
IC = N // P  # 8 chunks over i
ICP = IC // 2  # 4 DoubleRow pairs
JC = N // P  # 8 chunks over j
KC = DIN // P  # 2 chunks over k (wf compute)
G4 = 4  # rows per group
NG = ROWS // G4  # 32 groups per core

# how many of the 8 per-group X' ic-chunks formed by ACT ops (rest in one
# wide DVE broadcast op); each ACT chunk costs 4 activation ops
X_ACT_CHUNKS = 0

_DTYPE = "fp8_doublerow"  # informational (test.py prints it)

_CACHE = {}


def _build():
    import concourse.bass as bass
    import concourse.tile as tile
    from concourse import bacc, mybir
    from concourse._compat import axon_active
    from concourse.masks import make_identity

    f32 = mybir.dt.float32
    f32r = mybir.dt.float32r
    bf = mybir.dt.bfloat16
    f8 = mybir.dt.float8e4
    Copy = mybir.ActivationFunctionType.Copy
    DR = mybir.MatmulPerfMode.DoubleRow

    nc = bacc.Bacc(
        "TRN2",
        target_bir_lowering=False,
        debug=not axon_active(),
        num_devices=NCORES,
    )

    mh8_d = nc.dram_tensor("mh8", [N, N], f8, kind="ExternalInput").ap()
    mhb_d = nc.dram_tensor("mhb", [N, N], bf, kind="ExternalInput").ap()
    adjTc_d = nc.dram_tensor("adjTc", [N, ROWS], f32, kind="ExternalInput").ap()
    adjT_d = nc.dram_tensor("adjT", [N, ROWS], f32, kind="ExternalInput").ap()
    mfT_d = nc.dram_tensor("mfT", [N, ROWS], bf, kind="ExternalInput").ap()
    nfT_d = nc.dram_tensor("nfT", [DIN, N], f32, kind="ExternalInput").ap()
    w_d = nc.dram_tensor("w", [DIN, DOUT], f32, kind="ExternalInput").ap()
    out_d = nc.dram_tensor("out", [ROWS, DOUT], f32, kind="ExternalOutput").ap()

    with tile.TileContext(nc) as tc:
        with (
            tc.tile_pool(name="const", bufs=1) as cpool,
            tc.tile_pool(name="x", bufs=3) as xpool,
            tc.tile_pool(name="y", bufs=2) as ypool,
            tc.tile_pool(name="z", bufs=3) as zpool,
            tc.tile_pool(name="py", bufs=3, space="PSUM") as pypool,
            tc.tile_pool(name="pout", bufs=2, space="PSUM") as popool,
        ):
            # ---- resident tiles + input DMA (one multi-dim DMA per tensor;
            # per-DMA overhead dominates transfer time, so merge + order by
            # when compute needs them) ----
            nfT_sb = cpool.tile([P, KC * N], f32, tag="nfT")
            w_sb = cpool.tile([P, KC * DOUT], f32, tag="w")
            mhb_sb = cpool.tile([P, IC * N], bf, tag="mhb")
            mh8_sb = cpool.tile([P, IC * N], f8, tag="mh8")
            adjTc_sb = cpool.tile([P, N], f32, tag="adjTc")
            adjT_sb = cpool.tile([P, N], f32, tag="adjT")
            mfT_sb = cpool.tile([P, N], bf, tag="mfT")
            nc.sync.dma_start(
                w_sb[:].rearrange("p (kc d) -> p kc d", kc=KC),
                w_d.rearrange("(kc p) d -> p kc d", p=P),
            )
            nfT_v = nfT_sb[:].rearrange("p (kc c d) -> p kc c d", kc=KC, c=N // P)
            nfT_s = nfT_d.rearrange("(kc p) (c d) -> p kc c d", p=P, c=N // P)
            nc.sync.dma_start(nfT_v[:, :, : N // P // 2], nfT_s[:, :, : N // P // 2])
            nc.sync.dma_start(nfT_v[:, :, N // P // 2 :], nfT_s[:, :, N // P // 2 :])
            nc.sync.dma_start(
                adjTc_sb[:].rearrange("p (c r) -> p c r", c=N // P),
                adjTc_d.rearrange("(c p) r -> p c r", p=P),
            )
            nc.sync.dma_start(
                mh8_sb[:].rearrange("p (ic j) -> p ic j", ic=IC),
                mh8_d.rearrange("(ic p) j -> p ic j", p=P),
            )
            nc.sync.dma_start(
                mfT_sb[:].rearrange("p (c r) -> p c r", c=N // P),
                mfT_d.rearrange("(c p) r -> p c r", p=P),
            )
            nc.sync.dma_start(
                adjT_sb[:].rearrange("p (c r) -> p c r", c=N // P),
                adjT_d.rearrange("(c p) r -> p c r", p=P),
            )
            nc.sync.dma_start(
                mhb_sb[:].rearrange("p (ic j) -> p ic j", ic=IC),
                mhb_d.rearrange("(ic p) j -> p ic j", p=P),
            )

            # ---- setup compute ----
            # wf[n,d] = sum_k nf[n,k] w[k,d]; chunks of 128 n-rows (f32)
            wf_sb = cpool.tile([P, N], f32, tag="wf")
            for c in range(N // P):
                pt = pypool.tile([P, 1024], f32, tag="py")
                for kc in range(KC):
                    nc.tensor.matmul(
                        pt[:, :DOUT],
                        lhsT=nfT_sb[
                            :, (kc * (N // P) + c) * P : (kc * (N // P) + c + 1) * P
                        ],
                        rhs=w_sb[:, kc * DOUT : (kc + 1) * DOUT],
                        start=(kc == 0),
                        stop=(kc == KC - 1),
                    )
                nc.vector.tensor_copy(wf_sb[:, c * DOUT : (c + 1) * DOUT], pt[:, :DOUT])

            wf_bf = cpool.tile([P, N], bf, tag="wf_bf")
            nc.vector.tensor_copy(wf_bf[:], wf_sb[:])
            # wf4_bf: wf chunk jc replicated 4x along free dim, for Z = Ybf * wf
            wf4_bf = cpool.tile([P, JC * 512], bf, tag="wf4_bf")
            for jc in range(JC):
                for r in range(G4):
                    nc.vector.tensor_copy(
                        wf4_bf[:, jc * 512 + r * DOUT : jc * 512 + (r + 1) * DOUT],
                        wf_bf[:, jc * DOUT : (jc + 1) * DOUT],
                    )

            id_f32 = cpool.tile([P, P], f32, tag="id_f32")
            make_identity(nc, id_f32[:])
            id_bf = cpool.tile([P, P], bf, tag="id_bf")
            nc.vector.tensor_copy(id_bf[:], id_f32[:])

            # corrections: emitted after group 0's main matmuls so the PE
            # isn't stalled on the late mhb/adjT DMAs; C is first consumed
            # by matvec(0), which runs after main(1).
            C_sb = cpool.tile([P, DOUT], bf, tag="C")

            def emit_corrections():
                # Q'[j,d] = sum_i mh'[i,j] wf[i,d]  (bf16)
                Q_sb = cpool.tile([P, N], f32, tag="Q")
                for jc in range(JC):
                    pq = pypool.tile([P, 1024], f32, tag="py")
                    for ic in range(IC):
                        nc.tensor.matmul(
                            pq[:, :DOUT],
                            lhsT=mhb_sb[:, ic * N + jc * P : ic * N + (jc + 1) * P],
                            rhs=wf_bf[:, ic * DOUT : (ic + 1) * DOUT],
                            start=(ic == 0),
                            stop=(ic == IC - 1),
                        )
                    nc.scalar.activation(
                        Q_sb[:, jc * DOUT : (jc + 1) * DOUT], pq[:, :DOUT], Copy
                    )
                # Vq = wf .* Q'  (bf16)
                Vq_sb = cpool.tile([P, N], bf, tag="Vq")
                nc.vector.tensor_mul(Vq_sb[:], wf_bf[:], Q_sb[:])

                # S0 = adj @ wf (f32); mwf = mfs @ wf; G = mfs @ Vq
                s0_sb = cpool.tile([P, DOUT], f32, tag="s0")
                mwf_sb = cpool.tile([P, DOUT], f32, tag="mwf")
                g_sb = cpool.tile([P, DOUT], f32, tag="g")
                for dst, lhs_tile, rhs_tile in (
                    (s0_sb, adjT_sb, wf_sb),
                    (mwf_sb, mfT_sb, wf_bf),
                    (g_sb, mfT_sb, Vq_sb),
                ):
                    ps = pypool.tile([P, 1024], f32, tag="py")
                    for c in range(N // P):
                        nc.tensor.matmul(
                            ps[:, :DOUT],
                            lhsT=lhs_tile[:, c * P : (c + 1) * P],
                            rhs=rhs_tile[:, c * DOUT : (c + 1) * DOUT],
                            start=(c == 0),
                            stop=(c == N // P - 1),
                        )
                    nc.vector.tensor_copy(dst[:], ps[:, :DOUT])

                # C = 0.5*(S0 .* mwf + G)   (bf16)
                tmp_sb = cpool.tile([P, DOUT], f32, tag="tmpC")
                nc.vector.tensor_mul(tmp_sb[:], s0_sb[:], mwf_sb[:])
                nc.vector.tensor_add(tmp_sb[:], tmp_sb[:], g_sb[:])
                nc.vector.tensor_scalar_mul(C_sb[:], tmp_sb[:], 0.5)

            out_sb = cpool.tile([P, NG * DOUT], f32, tag="out_sb")

            # broadcast views for the X' formation (shared across groups)
            KD = IC - X_ACT_CHUNKS  # chunks covered by the wide DVE op
            wf_bc = (
                wf_sb[:, : KD * DOUT]
                .rearrange("p (ic d) -> p ic d", ic=KD)
                .unsqueeze(2)
                .broadcast_to([P, KD, G4, DOUT])
            )
            adj_r = adjTc_sb[:].rearrange("p (ic r) -> p ic r", ic=IC)

            def emit_matvec(b, z_t):
                # matvec: out[a,d] = sum_j mfs[a,j] Z[j,(s,d)]; 4-way col-tiled
                pout = popool.tile([P, 512], f32, tag="pout")
                for jc in range(JC):
                    for s in range(G4):
                        a = b * G4 + s
                        nc.tensor.matmul(
                            pout[32 * s : 32 * s + 1, :DOUT],
                            lhsT=mfT_sb[:, jc * P + a : jc * P + a + 1],
                            rhs=z_t[:, jc * 512 + s * DOUT : jc * 512 + (s + 1) * DOUT],
                            start=(jc == 0),
                            stop=False,
                            tile_position=(0, 32 * s),
                            skip_group_check=True,
                        )
                # += C[a,:] via one-hot identity column
                for s in range(G4):
                    a = b * G4 + s
                    nc.tensor.matmul(
                        pout[32 * s : 32 * s + 1, :DOUT],
                        lhsT=id_bf[:, a : a + 1],
                        rhs=C_sb[:],
                        start=False,
                        stop=True,
                        tile_position=(0, 32 * s),
                        skip_group_check=True,
                    )
                nc.scalar.activation(
                    out_sb[:, b * DOUT : (b + 1) * DOUT], pout[:, :DOUT], Copy
                )

            # ---- main loop: 32 groups of 4 rows; the matvec of group b is
            # issued after the main matmuls of group b+1 so the PE never
            # stalls on the ACT drain + DVE Z chain ----
            pending_mv = None
            for b in range(NG):
                # X'[i,(s,d)] = adj'[a,i] * wf[i,d]  -> fp8
                x_t = xpool.tile([P, IC * G4 * DOUT], f8, tag="X")
                if KD:
                    adj_bc = (
                        adj_r[:, :KD, G4 * b : G4 * (b + 1)]
                        .unsqueeze(3)
                        .broadcast_to([P, KD, G4, DOUT])
                    )
                    x_v = x_t[:, : KD * G4 * DOUT].rearrange(
                        "p (ic s d) -> p ic s d", ic=KD, s=G4
                    )
                    nc.vector.tensor_mul(x_v, wf_bc, adj_bc)
                for ic in range(KD, IC):
                    for s in range(G4):
                        a = b * G4 + s
                        nc.scalar.activation(
                            x_t[:, ic * 512 + s * DOUT : ic * 512 + (s + 1) * DOUT],
                            wf_sb[:, ic * DOUT : (ic + 1) * DOUT],
                            Copy,
                            scale=adjTc_sb[:, ic * P + a : ic * P + a + 1],
                        )

                # main matmul: Y'[j,(s,d)] accumulated over i-pairs (fp8 DoubleRow)
                ybf = ypool.tile([P, JC * 512], bf, tag="ybf")
                for jh in range(JC // 2):
                    py = pypool.tile([P, 1024], f32, tag="py")
                    for jl in range(2):
                        jc = jh * 2 + jl
                        for icp in range(ICP):
                            lhsT3 = mh8_sb[
                                :, 2 * icp * N : (2 * icp + 2) * N
                            ].rearrange("p (k f) -> p k f", k=2)[
                                :, :, jc * P : (jc + 1) * P
                            ]
                            rhs3 = x_t[
                                :, 2 * icp * 512 : (2 * icp + 2) * 512
                            ].rearrange("p (k f) -> p k f", k=2)
                            nc.tensor.matmul(
                                py[:, jl * 512 : (jl + 1) * 512],
                                lhsT=lhsT3,
                                rhs=rhs3,
                                start=(icp == 0),
                                stop=(icp == ICP - 1),
                                perf_mode=DR,
                            )
                    # drain 2 banks at once on ACT (f32 -> bf16)
                    nc.scalar.activation(
                        ybf[:, jh * 1024 : (jh + 1) * 1024], py[:], Copy
                    )
                if b == 0:
                    emit_corrections()
                if pending_mv is not None:
                    emit_matvec(*pending_mv)
                # Z = Ybf .* wf (bf16, DVE 2x)
                z_t = zpool.tile([P, JC * 512], bf, tag="Z")
                for h in range(2):
                    nc.vector.tensor_mul(
                        z_t[:, h * 2048 : (h + 1) * 2048],
                        ybf[:, h * 2048 : (h + 1) * 2048],
                        wf4_bf[:, h * 2048 : (h + 1) * 2048],
                    )
                pending_mv = (b, z_t)
                if b == NG // 2 + 1:
                    # first half of the output rows is complete -> start DMA
                    for s in range(G4):
                        nc.sync.dma_start(
                            out_d[s : 2 * ROWS // G4 : G4, :],
                            out_sb[32 * s : 32 * s + 1, : NG // 2 * DOUT],
                        )
            emit_matvec(*pending_mv)

            # ---- store: row 4b+s lives at out_sb[32s, b*128:(b+1)*128] ----
            for s in range(G4):
                nc.sync.dma_start(
                    out_d[2 * ROWS // G4 + s :: G4, :],
                    out_sb[32 * s : 32 * s + 1, NG // 2 * DOUT :],
                )

    nc.compile()
    return nc


def _prep_inputs(inputs):
    """Host-side sharding + layout prep. Returns per-core input maps."""
    import ml_dtypes

    bf16 = ml_dtypes.bfloat16
    f8 = ml_dtypes.float8_e4m3

    nf = np.asarray(inputs["node_features"], dtype=np.float32)
    adj = np.asarray(inputs["adjacency_matrix"], dtype=np.float32)
    mf = np.asarray(inputs["mask_father"], dtype=np.float32)[:, 0, :]
    ncnt = np.asarray(inputs["neighbor_count"], dtype=np.float32)
    mh = np.asarray(inputs["mask_hadamard"], dtype=np.float32)[:, 0, :]
    w = np.asarray(inputs["weight"], dtype=np.float32)

    mhp = mh - np.float32(0.5)
    mh8 = mhp.astype(f8)
    mhb = mhp.astype(bf16)
    mfs = mf / (ncnt * ncnt)  # fold 1/ncnt^2 into the father mask
    nfT = np.ascontiguousarray(nf.T)
    in_maps = []
    for c in range(NCORES):
        rows = slice(c * ROWS, (c + 1) * ROWS)
        adjr = adj[rows]
        in_maps.append(
            {
                "mh8": mh8,
                "mhb": mhb,
                "adjTc": np.ascontiguousarray(adjr.T) - np.float32(0.5),
                "adjT": np.ascontiguousarray(adjr.T),
                "mfT": np.ascontiguousarray(mfs[rows].T).astype(bf16),
                "nfT": nfT,
                "w": w,
            }
        )
    return in_maps


def _run(inputs, trace=False):
    from concourse import bass_utils

    if "nc" not in _CACHE:
        _CACHE["nc"] = _build()
    nc = _CACHE["nc"]
    in_maps = _prep_inputs(inputs)
    res = bass_utils.run_bass_kernel_spmd(
        nc, in_maps, core_ids=list(range(NCORES)), trace=trace
    )
    out = np.concatenate([r["out"] for r in res.results], axis=0)
    return out, res


def kernel(**inputs):
    out, _ = _run(inputs, trace=False)
    return out
